# revision 41
# baseline (speedup 1.0000x reference)
"""Trainium2 Bass kernel for channel-attention:
    scores[b,q,k] = sum_{h,w} Q[b,h,w,q] * K[b,h,w,k]
    attn = softmax_k(scores)
    out[b,h,w,q] = sum_k attn[b,q,k] * V[b,h,w,k]

Full inputs are [16, 128, 128, 64] f32. Data-parallel over batch across
8 NeuronCores (2 batches per core); no cross-core communication.

HBM-bound kernel: 16 MiB/core of DMA at ~350-430 GB/s union bandwidth
across the DMA rings dominates everything else, so the design is
organized entirely around keeping the rings streaming and hiding the
compute inside the load drain:

  - All device tensors 16-bit: Q, K, V fp16 (single-pass scores; abs
    score error ~0.05 vs ~45 typical top-2 gap -> rel err ~4e-3, well
    under the 2e-2 gate), out bf16.
  - Phase A packs two w-columns per matmul (lhsT = Q[h, 2w x 64q],
    rhs = K[h, 2w x 64k]); the [128,128] Gram accumulator's two
    diagonal 64x64 blocks sum to scores. fp16 [H,W,C] reinterprets as
    [H, W/2, 2C] with no host shuffle.
  - Softmax over the free dim, exp with accumulated row-sum on ACT,
    attn^T via PE transpose into a block-diagonal [128, 2, 64] fp16
    tile (bd). Phase C: one N=128 matmul per w-pair (lhsT = V^T pair
    from the host-transposed vt, rhs = bd) -> both output columns in
    PSUM; copied to bf16 on DVE; 32-w pieces stored as they complete.
  - Load order feeds the PE chain A0, T0, A1, T1, C0, C1 just in time:
    scalar ring [q0, q1, v0-hi, v1-tail], sync ring [k0, k1, v0-lo,
    v1-main]. Big pieces on purpose: HWDGE ring throughput drops
    sharply with DMA count (~227 GB/s at 4/ring vs ~195 at 7/ring).
    v1's tail piece is 0.5 MiB so only the last 16 pairs of C1 wait on
    the final load's completion receipt. Batch-0 stores (and b1's first
    two) ride the gpsimd SWDGE ring so they overlap the load drain;
    b1's last stores use the by-then-idle HWDGE rings, with the final
    piece split across both rings to halve the tail transfer.
  - tc.tile_wait_until fences pin the Tile scheduler's static per-
    engine order to that chain: its DMA cost model otherwise predicts
    different landing times and emits an order that stalls the PE ~11us
    on hardware.
"""

import sys

sys.path.insert(0, "/opt/trn_rl_repo")

import ml_dtypes
import numpy as np

_B, _H, _W, _C = 16, 128, 128, 64
_NCORES = 8
_BPC = _B // _NCORES  # batches per core
_PAIRS = _W // 2  # w-pairs per batch

_SPIECE = 32  # w-columns per output store piece (0.5 MiB bf16)
_NSP = _W // _SPIECE

# load piece boundaries, in w-pairs. Few big pieces: HWDGE ring rate
# drops sharply with DMA count (~227 GB/s at 4/ring vs ~195 at 7/ring),
# and a third concurrently-loading ring makes all rings slower (packet
# round-robin overhead), so loads stay on the two HWDGE rings.
_QK_PIECES = {0: [(0, 64)], 1: [(0, 64)]}
# v1's tail piece is small (0.5 MiB): it is the last load to land, and
# only the final 16 pairs of C1 wait on it. The 1.5/0.5 split also
# time-balances the rings against the scalar ring's ~2.7us late start.
_V_PIECES = {0: [(0, 32), (32, 64)], 1: [(0, 48), (48, 64)]}  # lo sync, hi scalar

_cache = {}


def _build_nc():
    from contextlib import ExitStack

    import concourse.bass as bass  # noqa: F401
    import concourse.tile as tile
    from concourse import bacc, mybir
    from concourse.masks import make_identity

    f32 = mybir.dt.float32
    f16 = mybir.dt.float16
    bf16 = mybir.dt.bfloat16
    nc = bacc.Bacc(target_bir_lowering=False)

    q_ext = nc.declare_dram_parameter(
        "q16", [_BPC, _H, _PAIRS, 2 * _C], f16, isOutput=False
    )
    k_ext = nc.declare_dram_parameter(
        "k16", [_BPC, _H, _PAIRS, 2 * _C], f16, isOutput=False
    )
    vt_ext = nc.declare_dram_parameter(
        "vt", [_BPC, 2 * _C, _PAIRS, _H], f16, isOutput=False
    )
    o_ext = nc.declare_dram_parameter("out", [_BPC, _H, _W, _C], bf16, isOutput=True)

    with tile.TileContext(nc) as tc, ExitStack() as ctx:
        singles = ctx.enter_context(tc.tile_pool(name="singles", bufs=1))
        qp = ctx.enter_context(tc.tile_pool(name="qp", bufs=2))
        kp = ctx.enter_context(tc.tile_pool(name="kp", bufs=2))
        vp = ctx.enter_context(tc.tile_pool(name="vp", bufs=4))
        op = ctx.enter_context(tc.tile_pool(name="op", bufs=8))
        sm = ctx.enter_context(tc.tile_pool(name="sm", bufs=2))
        ps_sc = ctx.enter_context(tc.tile_pool(name="ps_sc", bufs=2, space="PSUM"))
        ps_at = ctx.enter_context(tc.tile_pool(name="ps_at", bufs=2, space="PSUM"))
        ps_o = ctx.enter_context(tc.tile_pool(name="ps_o", bufs=4, space="PSUM"))

        # ---- loads: both HWDGE rings stream back-to-back, no store ever
        # ahead of a load in any ring's FIFO
        qt, kt, vt = {0: [], 1: []}, {0: [], 1: []}, {0: [], 1: []}
        for b in (0, 1):
            for i, (lo, hi) in enumerate(_QK_PIECES[b]):
                t = qp.tile([_H, hi - lo, 2 * _C], f16, tag="qt", name=f"qt{b}{i}")
                qt[b].append((t, lo, hi))
                t = kp.tile([_H, hi - lo, 2 * _C], f16, tag="kt", name=f"kt{b}{i}")
                kt[b].append((t, lo, hi))
            for i, (lo, hi) in enumerate(_V_PIECES[b]):
                t = vp.tile([2 * _C, hi - lo, _H], f16, tag="vt", name=f"vt{b}{i}")
                vt[b].append((t, lo, hi))

        for b in (0, 1):
            for t, lo, hi in qt[b]:
                nc.scalar.dma_start(out=t, in_=q_ext[b, :, lo:hi, :])
            for t, lo, hi in kt[b]:
                nc.sync.dma_start(out=t, in_=k_ext[b, :, lo:hi, :])
        for b in (0, 1):
            t, lo, hi = vt[b][0]
            nc.sync.dma_start(out=t, in_=vt_ext[b, :, lo:hi, :])
        for b in (0, 1):
            t, lo, hi = vt[b][1]
            nc.scalar.dma_start(out=t, in_=vt_ext[b, :, lo:hi, :])

        ident = singles.tile([_C, _C], f32)
        make_identity(nc, ident)

        def emit_phase_a(b):
            gram = ps_sc.tile([2 * _C, 2, _C], f32, tag="gram")
            for t, lo, hi in qt[b]:
                kt_t = next(x[0] for x in kt[b] if x[1] == lo)
                for jj in range(hi - lo):
                    j = lo + jj
                    nc.tensor.matmul(
                        gram,
                        lhsT=t[:, jj, :],
                        rhs=kt_t[:, jj, :],
                        start=(j == 0),
                        stop=(j == _PAIRS - 1),
                    )
            return gram

        def emit_softmax(gram):
            # scores = even-w block + odd-w block of the pair Gram tile
            s0 = sm.tile([_C, _C], f32, tag="s0")
            nc.vector.tensor_copy(out=s0, in_=gram[0:_C, 0, :])
            scores = sm.tile([_C, _C], f32, tag="scores")
            nc.vector.tensor_tensor(
                out=scores,
                in0=gram[_C : 2 * _C, 1, :],
                in1=s0,
                op=mybir.AluOpType.add,
            )
            negmax = sm.tile([_C, 1], f32, tag="negmax")
            nc.vector.tensor_reduce(
                out=negmax,
                in_=scores,
                axis=mybir.AxisListType.X,
                op=mybir.AluOpType.max,
                negate=True,
            )
            e = sm.tile([_C, _C], f32, tag="e")
            ssum = sm.tile([_C, 1], f32, tag="ssum")
            nc.scalar.activation(
                out=e,
                in_=scores,
                func=mybir.ActivationFunctionType.Exp,
                bias=negmax,
                scale=1.0,
                accum_out=ssum,
            )
            rsum = sm.tile([_C, 1], f32, tag="rsum")
            nc.vector.reciprocal(out=rsum, in_=ssum)
            attn = sm.tile([_C, _C], f32, tag="attn")
            nc.vector.tensor_scalar_mul(attn, e, rsum)
            return attn

        def emit_bd(attn):
            # attn^T via PE transpose + block-diagonal fp16 tile. Kept
            # separate from emit_softmax so batch 1's transpose can be
            # fenced AFTER C0: on the PE it otherwise sits between A1
            # and C0 and adds the softmax-chain latency to C0's start.
            attnT_ps = ps_at.tile([_C, _C], f32, tag="attnT_ps")
            nc.tensor.transpose(attnT_ps, attn, ident)
            bd = sm.tile([2 * _C, 2, _C], f16, tag="bd")
            nc.vector.memset(bd, 0.0)
            nc.vector.tensor_copy(out=bd[0:_C, 0, :], in_=attnT_ps)
            nc.vector.tensor_copy(out=bd[_C : 2 * _C, 1, :], in_=attnT_ps)
            return bd

        def emit_phase_c(b, bd, store_rings):
            ppp = _SPIECE // 2  # w-pairs per store piece
            for pc in range(_NSP):
                otile = op.tile([_H, _SPIECE, _C], bf16, tag="otile")
                for wg in range(0, ppp, 4):  # 4 pairs per PSUM bank
                    o_ps = ps_o.tile([_H, 8, _C], f32, tag="o_ps")
                    for half in range(4):
                        j = pc * ppp + wg + half
                        vt_t, lo, hi = next(x for x in vt[b] if x[1] <= j < x[2])
                        nc.tensor.matmul(
                            o_ps[:, 2 * half : 2 * half + 2, :],
                            lhsT=vt_t[:, j - lo, :],
                            rhs=bd,
                            start=True,
                            stop=True,
                        )
                    # alternate DVE/ACT so piece production keeps pace
                    # with the PE (one engine alone is slower than C's
                    # matmul stream and would throttle it via PSUM reuse)
                    if (wg // 4 + pc) % 2 == 0:
                        nc.vector.tensor_copy(
                            out=otile[:, 2 * wg : 2 * wg + 8, :], in_=o_ps
                        )
                    else:
                        nc.scalar.activation(
                            out=otile[:, 2 * wg : 2 * wg + 8, :],
                            in_=o_ps,
                            func=mybir.ActivationFunctionType.Copy,
                        )
                w0 = pc * _SPIECE
                rings = store_rings[pc]
                if not isinstance(rings, list):
                    rings = [rings]
                # split the piece across the given rings (tail latency)
                wstep = _SPIECE // len(rings)
                for ri, eng in enumerate(rings):
                    sl = slice(w0 + ri * wstep, w0 + (ri + 1) * wstep)
                    eng.dma_start(
                        out=o_ext[b, :, sl, :],
                        in_=otile[:, ri * wstep : (ri + 1) * wstep, :],
                    )

        # Fences pin the scheduler's static per-engine order to the chain
        # A0, T0, A1, C0, T1, C1 (see module docstring). T1/bd1 go after
        # C0's matmuls: there they hide inside C1's v1-landing wait
        # instead of delaying C0's start by the softmax-chain latency.
        with tc.tile_wait_until(0.000):
            gram0 = emit_phase_a(0)
        with tc.tile_wait_until(0.010):
            bd0 = emit_bd(emit_softmax(gram0))
        with tc.tile_wait_until(0.020):
            gram1 = emit_phase_a(1)
        with tc.tile_wait_until(0.030):
            attn1 = emit_softmax(gram1)
        with tc.tile_wait_until(0.040):
            emit_phase_c(0, bd0, [nc.gpsimd] * 4)
        with tc.tile_wait_until(0.045):
            bd1 = emit_bd(attn1)
        with tc.tile_wait_until(0.050):
            emit_phase_c(
                1, bd1, [nc.gpsimd, nc.gpsimd, nc.scalar, [nc.sync, nc.scalar]]
            )

    nc.finalize()
    return nc


def _get_nc():
    if "nc" not in _cache:
        _cache["nc"] = _build_nc()
    return _cache["nc"]


def _prep_inputs(q, k, v):
    """Host-side layout prep: fp16 casts, V transposed per w-pair."""
    q16 = q.astype(np.float16)  # [B, H, W, C] == [B, H, W/2, 2C]
    k16 = k.astype(np.float16)
    v16 = v.astype(np.float16)  # [B, H, W, C]
    # vt[b, (dw c), j, h] = v[b, h, 2j+dw, c]
    x = v16.transpose(0, 2, 3, 1)  # [B, W, C, H]
    x = x.reshape(_B, _PAIRS, 2, _C, _H)  # [B, j, dw, C, H]
    vt = np.ascontiguousarray(x.transpose(0, 2, 3, 1, 4)).reshape(
        _B, 2 * _C, _PAIRS, _H
    )
    q16 = q16.reshape(_B, _H, _PAIRS, 2 * _C)
    k16 = k16.reshape(_B, _H, _PAIRS, 2 * _C)
    return q16, k16, vt


def run(inputs, trace=False):
    """Run the SPMD kernel. Returns (full_output, BassKernelResults)."""
    from concourse.bass_utils import run_bass_kernel_spmd

    q = np.asarray(inputs["query"], dtype=np.float32)
    k = np.asarray(inputs["keys"], dtype=np.float32)
    v = np.asarray(inputs["values"], dtype=np.float32)
    assert q.shape == (_B, _H, _W, _C), q.shape

    q16, k16, vt = _prep_inputs(q, k, v)

    nc = _get_nc()
    in_maps = []
    for i in range(_NCORES):
        sl = slice(i * _BPC, (i + 1) * _BPC)
        in_maps.append({"q16": q16[sl], "k16": k16[sl], "vt": vt[sl]})

    res = run_bass_kernel_spmd(
        nc, in_maps, core_ids=list(range(_NCORES)), trace=trace
    )
    out = np.concatenate(
        [res.results[i]["out"].astype(np.float32) for i in range(_NCORES)], axis=0
    )
    return out, res


def kernel(**inputs) -> np.ndarray:
    out, _ = run(inputs, trace=False)
    return out
